# revision 4
# baseline (speedup 1.0000x reference)
"""LoRA layer kernel for Trainium2, 8-core data-parallel.

out = x @ W.T + 2.0 * ((x @ B) @ A) = x @ (W.T + 2*(B@A)) = x @ Weff

The LoRA path is folded into the weight on the HOST (B@A is a tiny
rank-16 outer product) so the device kernel is a single dense GEMM:
out[16384, 4096] = x[16384, 4096] @ Weff[4096, 4096].

Sharding: data-parallel over rows, 2048 rows/core, Weff replicated.

Per-core kernel: operands in bf16 (tolerance 2e-2; bf16 gives ~2e-3).
Rows in two resident blocks of 1024 (8 m-tiles); for each block, 8
output chunks of 512 (one PSUM bank each) accumulate over 32 k-tiles.
x and W are pre-tiled on host into [128, N]-contiguous 1MiB chunks so
HWDGE/SWDGE descriptor generation (which saturated the sync sequencer
in earlier versions) is trivial. Output DMAs alternate sync/scalar;
PSUM evictions alternate vector/scalar. 4096 MMs of [128x128x512]
@ ~216ns = ~884us PE floor.
"""

import sys

if "/opt/trn_rl_repo" not in sys.path:
    sys.path.insert(0, "/opt/trn_rl_repo")

import os

import numpy as np
import ml_dtypes

import concourse.bass as bass
import concourse.mybir as mybir
import concourse.tile as tile

N_CORES = 8
D = 4096
ROWS_TOTAL = 4 * 4096          # 16384
ROWS_PER_CORE = ROWS_TOTAL // N_CORES  # 2048
P = 128
KT = D // P                    # 32 k-tiles
M_BLOCK = 1024                 # rows per x-resident block
N_BLOCKS = ROWS_PER_CORE // M_BLOCK    # 2
MT_PER_BLOCK = M_BLOCK // P    # 8 m-tiles (PSUM banks)
OC = 512                       # o-chunk width (one PSUM bank)
N_OC = D // OC                 # 8
KH = KT // 2                   # k-tiles per x half-tile
KQ = 4                         # k-tiles per DMA chunk
N_KQ = KT // KQ                # 8 chunks

F32 = mybir.dt.float32
BF16 = mybir.dt.bfloat16

WARMUP = os.environ.get("K_WARMUP", "1") == "1"


def split_wide_waits(nc, max_waits=1):
    """walrus in this container rejects >1 sync wait per instruction;
    move excess waits onto preceding same-engine NoOps."""
    n_split = 0
    for f in nc.m.functions:
        for bb in f.blocks:
            new_insts = []
            for inst in bb.instructions:
                si = getattr(inst, "sync_info", None)
                if si is not None and si.on_wait and len(si.on_wait) > max_waits:
                    waits = list(si.on_wait)
                    keep = waits[-max_waits:]
                    extra = waits[:-max_waits]
                    for i in range(0, len(extra), max_waits):
                        chunk = extra[i:i + max_waits]
                        nop = mybir.InstNoOp(
                            name=f"{inst.name}_wsplit{i}",
                            sync_info=mybir.SyncInfo(on_wait=chunk, on_update=[]),
                            bass_nofuse=True,
                            engine=inst.engine,
                        )
                        new_insts.append(nop)
                        n_split += 1
                    si.on_wait = keep
                new_insts.append(inst)
            bb.instructions[:] = new_insts
    return n_split


def build_program():
    nc = bass.Bass()
    # xq: pre-tiled x, rows (blk*8+kq)*128.. hold chunk [128, 4*1024]
    xq = nc.declare_dram_parameter("xq", [N_BLOCKS * N_KQ * P, KQ * M_BLOCK], BF16, isOutput=False)
    # wq: pre-tiled Weff, rows (oc*8+kq)*128.. hold chunk [128, 4*512]
    wq = nc.declare_dram_parameter("wq", [N_OC * N_KQ * P, KQ * OC], BF16, isOutput=False)
    cz = nc.declare_dram_parameter("cz", [P, OC], BF16, isOutput=False)
    out = nc.declare_dram_parameter("out", [ROWS_PER_CORE, D], F32, isOutput=True)

    with tile.TileContext(nc) as tc:
        with (
            tc.tile_pool(name="xpool_a", bufs=2) as xpool_a,
            tc.tile_pool(name="xpool_b", bufs=2) as xpool_b,
            tc.tile_pool(name="wpool", bufs=8) as wpool,
            tc.tile_pool(name="opool", bufs=4) as opool,
            tc.tile_pool(name="cpool", bufs=1) as cpool,
            tc.tile_pool(name="ppool", bufs=8, space="PSUM") as ppool,
        ):
            # zeros tile for HAM warmup matmuls
            ztile = cpool.tile([P, OC], BF16, tag="zt")
            nc.sync.dma_start(ztile[:], cz[:])

            # HAM warmup: dummy matmuls so the PE clock is at 8/8 before
            # real work lands (overlaps the DMA-descriptor startup window).
            if WARMUP:
                junk = ppool.tile([P, OC], F32, tag="acc", name="junk")
                for i in range(22):
                    nc.tensor.matmul(
                        junk[:],
                        ztile[:, :P],
                        ztile[:],
                        start=(i == 0),
                        stop=(i == 21),
                    )

            for blk in range(N_BLOCKS):
                r0 = blk * M_BLOCK
                # x block resident: two half tiles (k 0-15, k 16-31)
                xa = xpool_a.tile([P, KH * M_BLOCK], BF16, tag="xa")
                xb = xpool_b.tile([P, KH * M_BLOCK], BF16, tag="xb")

                def xsl(k, c0, cw):
                    t = xa if k < KH else xb
                    kk = k % KH
                    return t[:, kk * M_BLOCK + c0: kk * M_BLOCK + c0 + cw]

                # load x block as 8 contiguous 1MiB chunks (4 k-tiles each),
                # alternating gpsimd/scalar queues
                for kq in range(N_KQ):
                    t = xa if kq < N_KQ // 2 else xb
                    q0 = (kq % (N_KQ // 2)) * KQ * M_BLOCK
                    eng = nc.gpsimd if kq % 2 == 0 else nc.scalar
                    rr = (blk * N_KQ + kq) * P
                    eng.dma_start(t[:, q0:q0 + KQ * M_BLOCK], xq[rr:rr + P, :])

                # main GEMM: W fetched as contiguous [128, 4*512] chunks
                for oc in range(N_OC):
                    psums = []
                    for mt in range(MT_PER_BLOCK):
                        psums.append(ppool.tile([P, OC], F32, tag="acc", name=f"ps_{blk}_{oc}_{mt}"))
                    for kq in range(N_KQ):
                        wtile = wpool.tile([P, KQ * OC], BF16, tag="wt")
                        rr = (oc * N_KQ + kq) * P
                        nc.sync.dma_start(wtile[:], wq[rr:rr + P, :])
                        for kk in range(KQ):
                            k = KQ * kq + kk
                            for mt in range(MT_PER_BLOCK):
                                nc.tensor.matmul(
                                    psums[mt][:],
                                    xsl(k, mt * P, P),
                                    wtile[:, kk * OC:(kk + 1) * OC],
                                    start=(k == 0),
                                    stop=(k == KT - 1),
                                )
                    for mt in range(MT_PER_BLOCK):
                        ot = opool.tile([P, OC], F32, tag="ot")
                        if mt % 2 == 0:
                            nc.vector.tensor_copy(ot[:], psums[mt][:])
                        else:
                            nc.scalar.copy(ot[:], psums[mt][:])
                        deng = nc.sync if mt % 2 == 0 else nc.scalar
                        deng.dma_start(
                            out[r0 + mt * P:r0 + (mt + 1) * P,
                                oc * OC:(oc + 1) * OC],
                            ot[:],
                        )

    split_wide_waits(nc)
    return nc


_NC_CACHE = [None]


def _pretile_w(weff_bf):
    # [4096, 4096] -> [8 oc, 8 kq, 128 p, 4 kk, 512 c] -> [8192, 2048]
    w = weff_bf.reshape(N_KQ, KQ, P, N_OC, OC)          # kq, kk, p, oc, c
    w = w.transpose(3, 0, 2, 1, 4)                      # oc, kq, p, kk, c
    return np.ascontiguousarray(w).reshape(N_OC * N_KQ * P, KQ * OC)


def _pretile_x(xt_c):
    # xt_c: [4096, 2048] (k-major, rows for this core transposed)
    # -> [2 blk, 8 kq, 128 p, 4 q, 1024 m] -> [2048, 4096]
    xv = xt_c.reshape(N_KQ, KQ, P, N_BLOCKS, M_BLOCK)   # kq, q, p, blk, m
    xv = xv.transpose(3, 0, 2, 1, 4)                    # blk, kq, p, q, m
    return np.ascontiguousarray(xv).reshape(N_BLOCKS * N_KQ * P, KQ * M_BLOCK)


def kernel(x, weight, lora_A, lora_B):
    from concourse.bass_utils import run_bass_kernel_spmd

    x = np.asarray(x, dtype=np.float32)
    weight = np.asarray(weight, dtype=np.float32)
    lora_A = np.asarray(lora_A, dtype=np.float32)
    lora_B = np.asarray(lora_B, dtype=np.float32)

    # fold LoRA into the weight: out = x @ (W.T + 2*(B@A))
    weff = weight.T + 2.0 * (lora_B @ lora_A)
    wq = _pretile_w(weff.astype(ml_dtypes.bfloat16))

    x2 = x.reshape(ROWS_TOTAL, D).astype(ml_dtypes.bfloat16)
    cz = np.zeros((P, OC), dtype=ml_dtypes.bfloat16)

    in_maps = []
    for c in range(N_CORES):
        xt_c = np.ascontiguousarray(
            x2[c * ROWS_PER_CORE:(c + 1) * ROWS_PER_CORE].T
        )
        in_maps.append({"xq": _pretile_x(xt_c), "wq": wq, "cz": cz})

    if _NC_CACHE[0] is None:
        _NC_CACHE[0] = build_program()
    nc = _NC_CACHE[0]

    res = run_bass_kernel_spmd(nc, in_maps, list(range(N_CORES)))
    out = np.concatenate(
        [res.results[c]["out"] for c in range(N_CORES)], axis=0
    )
    return out.reshape(x.shape)
